# revision 5
# baseline (speedup 1.0000x reference)
"""Trainium2 Bass kernel for segment-mean + 2-layer MLP with training-mode BatchNorm.

Reference computation (see harness):
    ends = cumsum(length); seg_ids = searchsorted(ends, arange(N), 'right')
    mean  = segment_sum(x, seg_ids, B) / length[:, None]          # [512, 32]
    h   = relu(BN(mean @ W1 + b1, g1, beta1))                     # BN over batch dim
    out = BN(h @ W2 + b2, g2, beta2)                              # [512, 128]

Strategy (8 NeuronCores, full inputs in / full output out):
  Launch A (SPMD x8, memory-bound part):
    - 512 segments are rank-sorted by length and dealt into 64 "slots" x 8 cores
      (slot i holds the 8 segments ranked [8i, 8i+8); one per core), so every
      core runs the IDENTICAL program on a same-shape buffer.
    - Host packs each core's buffer [128, W]: slot i padded to L_i rows
      (L_i = per-slot max, multiple of 128, same on all cores); partition p
      holds rows [p*L_i/128, (p+1)*L_i/128) of the slot, so each slot occupies
      the same free-dim window on every partition.
    - Device: stream slot-aligned tiles (contiguous per-partition DMA), one
      strided VectorE reduce per slot ([128,(r c)] -> [128, 32]), then a
      single ones-vector TensorE matmul merges the 128 partitions into
      [1, 64*32], scaled by 1/len -> per-slot means.
  Launch B (1 core): tiny MLP+BN on the gathered [512, 32] means. Batch lives
    on the free axis (h^T layouts) so BN stats are free-axis reduces and the
    normalization is one fused scalar-engine activation per layer.

kernel() is self-contained: shapes/sharding hardcoded, no file reads.
"""

import sys

if "/opt/trn_rl_repo" not in sys.path:
    sys.path.insert(0, "/opt/trn_rl_repo")

import numpy as np

import concourse.bass as bass
import concourse.tile as tile
from concourse import bacc, mybir
from concourse.bass_utils import run_bass_kernel_spmd

F32 = mybir.dt.float32

N_TOTAL = 4_194_304
B = 512
C_IN = 32
FC1 = 64
FC2 = 128
EPS = 1e-5
N_CORES = 8
P = 128
SLOTS = B // N_CORES          # 64 slots per core
TILE_W = 4096                 # target free-dim elems per DMA tile (16 KiB/partition)


# ---------------------------------------------------------------- host layout

def _plan(lens):
    """Assign segments to (core, slot), pick padded slot lengths and DMA tiles.

    Returns dict with:
      seg_of[c][i] -> segment id
      li[i]        -> rows per partition for slot i (L_i = 128*li)
      w[i]         -> free-dim elems per slot (li*32)
      tiles        -> list of (offset, width, [(slot, off_in_tile, li), ...])
    """
    order = np.argsort(-lens, kind="stable")
    seg_of = np.empty((N_CORES, SLOTS), dtype=np.int64)
    li = np.empty(SLOTS, dtype=np.int64)
    for i in range(SLOTS):
        group = order[i * N_CORES:(i + 1) * N_CORES]
        seg_of[:, i] = group
        li[i] = (int(lens[group].max()) + P - 1) // P
    w = li * C_IN
    tiles = []
    cur = []
    cur_w = 0
    off = 0
    for i in range(SLOTS):
        if cur and cur_w + int(w[i]) > TILE_W:
            tiles.append((off, cur_w, cur))
            off += cur_w
            cur, cur_w = [], 0
        cur.append((i, cur_w, int(li[i])))
        cur_w += int(w[i])
    if cur:
        tiles.append((off, cur_w, cur))
    max_w = max(t[1] for t in tiles)
    return {"seg_of": seg_of, "li": li, "w": w, "W": int(w.sum()),
            "tiles": tiles, "max_w": max_w}


def _pack(x, lens, starts, plan):
    """Build per-core device buffers [128, W] plus per-core 1/len rows."""
    W = plan["W"]
    seg_of = plan["seg_of"]
    li = plan["li"]
    xbufs = []
    invs = []
    for c in range(N_CORES):
        buf = np.zeros((P, W), dtype=np.float32)
        off = 0
        for i in range(SLOTS):
            s = int(seg_of[c, i])
            L, wi = int(lens[s]), int(li[i] * C_IN)
            rows = np.zeros((P * int(li[i]), C_IN), dtype=np.float32)
            rows[:L] = x[starts[s]:starts[s] + L]
            buf[:, off:off + wi] = rows.reshape(P, wi)
            off += wi
        xbufs.append(buf)
        linv = (np.float32(1.0) / lens[seg_of[c]].astype(np.float32))
        invs.append(np.repeat(linv, C_IN)[None, :].astype(np.float32))
    return xbufs, invs


# ---------------------------------------------------------------- device progs

def _build_a(plan):
    """Launch A: per-core segment means -> [1, SLOTS*C_IN]."""
    W = plan["W"]
    nc = bacc.Bacc("TRN2", target_bir_lowering=False, debug=False)
    x_d = nc.dram_tensor("xd", [P, W], F32, kind="ExternalInput")
    inv_d = nc.dram_tensor("inv", [1, SLOTS * C_IN], F32, kind="ExternalInput")
    out_d = nc.dram_tensor("means_flat", [1, SLOTS * C_IN], F32, kind="ExternalOutput")

    with tile.TileContext(nc) as tc:
        with (
            tc.tile_pool(name="xin", bufs=3) as xin,
            tc.tile_pool(name="cons", bufs=1) as cons,
            tc.tile_pool(name="ps", bufs=1, space="PSUM") as ps,
        ):
            partials = cons.tile([P, SLOTS * C_IN], F32)
            ones = cons.tile([P, 1], F32)
            nc.vector.memset(ones[:], 1.0)
            inv = cons.tile([1, SLOTS * C_IN], F32)
            nc.sync.dma_start(inv[:], inv_d[:])

            for off, wt, slots in plan["tiles"]:
                t = xin.tile([P, plan["max_w"]], F32, tag="xtile")
                nc.sync.dma_start(t[:, :wt], x_d[:, off:off + wt])
                for i, soff, sli in slots:
                    src = t[:, soff:soff + sli * C_IN].rearrange(
                        "p (r c) -> p c r", c=C_IN)
                    nc.vector.reduce_sum(
                        partials[:, i * C_IN:(i + 1) * C_IN], src,
                        axis=mybir.AxisListType.X)

            acc = ps.tile([1, SLOTS * C_IN], F32)
            for k in range(0, SLOTS * C_IN, 512):
                ke = min(k + 512, SLOTS * C_IN)
                nc.tensor.matmul(acc[:, k:ke], ones[:], partials[:, k:ke],
                                 start=True, stop=True)
            means = cons.tile([1, SLOTS * C_IN], F32)
            nc.vector.tensor_mul(means[:], acc[:, :], inv[:])
            nc.sync.dma_start(out_d[:], means[:])
    nc.compile()
    return nc


def _bn_free_axis(nc, cons, sb, h_sb, n_par, n_free, b_t, g_t, bt_t, relu):
    """BN over the free axis of h_sb [n_par, n_free] (+ optional ReLU), in place.

    h_sb already contains (pre-activation + bias). Returns output tile.
    """
    zeros1 = cons.tile([P, 1], F32, tag="zeros1")
    nc.vector.memset(zeros1[:], 0.0)
    eps1 = cons.tile([P, 1], F32, tag="eps1")
    nc.vector.memset(eps1[:], EPS)

    rs = sb.tile([n_par, 1], F32, tag=f"rs{n_par}")
    sq = sb.tile([n_par, n_free], F32, tag=f"sq{n_par}")
    ssq = sb.tile([n_par, 1], F32, tag=f"ssq{n_par}")
    nc.scalar.activation(sq[:], h_sb[:], mybir.ActivationFunctionType.Square,
                         bias=zeros1[0:n_par, :], accum_out=ssq[:])
    nc.vector.reduce_sum(rs[:], h_sb[:], axis=mybir.AxisListType.X)

    mu = sb.tile([n_par, 1], F32, tag=f"mu{n_par}")
    nc.scalar.mul(mu[:], rs[:], 1.0 / n_free)
    var = sb.tile([n_par, 1], F32, tag=f"var{n_par}")
    nc.scalar.mul(var[:], ssq[:], 1.0 / n_free)
    musq = sb.tile([n_par, 1], F32, tag=f"musq{n_par}")
    nc.vector.tensor_mul(musq[:], mu[:], mu[:])
    nc.vector.tensor_sub(var[:], var[:], musq[:])
    std = sb.tile([n_par, 1], F32, tag=f"std{n_par}")
    nc.scalar.activation(std[:], var[:], mybir.ActivationFunctionType.Sqrt,
                         bias=eps1[0:n_par, :], scale=1.0)
    rstd = sb.tile([n_par, 1], F32, tag=f"rstd{n_par}")
    nc.vector.reciprocal(rstd[:], std[:])

    scale = sb.tile([n_par, 1], F32, tag=f"scale{n_par}")
    nc.vector.tensor_mul(scale[:], g_t[:], rstd[:])
    bias = sb.tile([n_par, 1], F32, tag=f"bias{n_par}")
    nc.vector.tensor_mul(bias[:], mu[:], scale[:])
    nc.vector.tensor_sub(bias[:], bt_t[:], bias[:])

    out = sb.tile([n_par, n_free], F32, tag=f"bnout{n_par}")
    func = (mybir.ActivationFunctionType.Relu if relu
            else mybir.ActivationFunctionType.Identity)
    # activation computes func(in*scale + bias): scale = g*rstd,
    # bias = beta - mu*g*rstd  ->  func(g*(h-mu)*rstd + beta)
    nc.scalar.activation(out[:], h_sb[:], func, bias=bias[:], scale=scale[:])
    return out


def _build_b():
    """Launch B: [512, 32] means -> MLP+BN -> [512, 128]."""
    nc = bacc.Bacc("TRN2", target_bir_lowering=False, debug=False)
    m_d = nc.dram_tensor("means", [B, C_IN], F32, kind="ExternalInput")
    id_d = nc.dram_tensor("ident", [P, P], F32, kind="ExternalInput")
    w1_d = nc.dram_tensor("w1", [C_IN, FC1], F32, kind="ExternalInput")
    w2_d = nc.dram_tensor("w2", [FC1, FC2], F32, kind="ExternalInput")
    b1_d = nc.dram_tensor("b1", [FC1, 1], F32, kind="ExternalInput")
    g1_d = nc.dram_tensor("g1", [FC1, 1], F32, kind="ExternalInput")
    bt1_d = nc.dram_tensor("bt1", [FC1, 1], F32, kind="ExternalInput")
    b2_d = nc.dram_tensor("b2", [FC2, 1], F32, kind="ExternalInput")
    g2_d = nc.dram_tensor("g2", [FC2, 1], F32, kind="ExternalInput")
    bt2_d = nc.dram_tensor("bt2", [FC2, 1], F32, kind="ExternalInput")
    out_d = nc.dram_tensor("out", [B, FC2], F32, kind="ExternalOutput")

    with tile.TileContext(nc) as tc:
        with (
            tc.tile_pool(name="cons", bufs=1) as cons,
            tc.tile_pool(name="sb", bufs=1) as sb,
            tc.tile_pool(name="pst", bufs=2, space="PSUM") as pst,
            tc.tile_pool(name="psm", bufs=1, space="PSUM") as psm,
        ):
            ident = cons.tile([P, P], F32)
            nc.sync.dma_start(ident[:], id_d[:])
            w1 = cons.tile([C_IN, FC1], F32)
            nc.sync.dma_start(w1[:], w1_d[:])
            w2 = cons.tile([FC1, FC2], F32)
            nc.sync.dma_start(w2[:], w2_d[:])
            b1 = cons.tile([FC1, 1], F32)
            nc.sync.dma_start(b1[:], b1_d[:])
            g1 = cons.tile([FC1, 1], F32)
            nc.sync.dma_start(g1[:], g1_d[:])
            bt1 = cons.tile([FC1, 1], F32)
            nc.sync.dma_start(bt1[:], bt1_d[:])
            b2 = cons.tile([FC2, 1], F32)
            nc.sync.dma_start(b2[:], b2_d[:])
            g2 = cons.tile([FC2, 1], F32)
            nc.sync.dma_start(g2[:], g2_d[:])
            bt2 = cons.tile([FC2, 1], F32)
            nc.sync.dma_start(bt2[:], bt2_d[:])

            # means^T [32, 512] via 4 tensor-engine transposes
            mt = sb.tile([C_IN, B], F32)
            for j in range(B // P):
                mj = sb.tile([P, C_IN], F32, tag="mj")
                nc.sync.dma_start(mj[:], m_d[j * P:(j + 1) * P, :])
                tp = pst.tile([C_IN, P], F32, tag="tp")
                nc.tensor.transpose(tp[:], mj[:], ident[:])
                nc.scalar.copy(mt[:, j * P:(j + 1) * P], tp[:])

            # layer 1: h1^T [64, 512] = W1^T @ means^T, +b1, BN, ReLU
            h1_ps = psm.tile([FC1, B], F32, tag="h1")
            nc.tensor.matmul(h1_ps[:], w1[:], mt[:], start=True, stop=True)
            h1 = sb.tile([FC1, B], F32)
            nc.scalar.activation(h1[:], h1_ps[:],
                                 mybir.ActivationFunctionType.Identity,
                                 bias=b1[:])
            a1 = _bn_free_axis(nc, cons, sb, h1, FC1, B, b1, g1, bt1,
                               relu=True)

            # layer 2: h2^T [128, 512] = W2^T @ a1, +b2, BN
            h2_ps = psm.tile([FC2, B], F32, tag="h2")
            nc.tensor.matmul(h2_ps[:], w2[:], a1[:], start=True, stop=True)
            h2 = sb.tile([FC2, B], F32)
            nc.scalar.activation(h2[:], h2_ps[:],
                                 mybir.ActivationFunctionType.Identity,
                                 bias=b2[:])
            o = _bn_free_axis(nc, cons, sb, h2, FC2, B, b2, g2, bt2,
                              relu=False)

            # transpose back to [512, 128] and store
            for j in range(B // P):
                tp2 = pst.tile([P, P], F32, tag="tp2")
                nc.tensor.transpose(tp2[:], o[:, j * P:(j + 1) * P], ident[:])
                ob = sb.tile([P, P], F32, tag="ob")
                nc.scalar.copy(ob[:], tp2[:])
                nc.sync.dma_start(out_d[j * P:(j + 1) * P, :], ob[:])
    nc.compile()
    return nc


# ---------------------------------------------------------------- entry point

def _run(inputs, trace=False):
    x = np.ascontiguousarray(np.asarray(inputs["x"], dtype=np.float32))
    lens = np.asarray(inputs["length"]).astype(np.int64)
    starts = np.zeros(B + 1, dtype=np.int64)
    np.cumsum(lens, out=starts[1:])
    assert starts[-1] == x.shape[0]

    plan = _plan(lens)
    xbufs, invs = _pack(x, lens, starts, plan)

    nc_a = _build_a(plan)
    in_maps = [{"xd": xbufs[c], "inv": invs[c]} for c in range(N_CORES)]
    res_a = run_bass_kernel_spmd(nc_a, in_maps, list(range(N_CORES)),
                                 trace=trace)

    means = np.empty((B, C_IN), dtype=np.float32)
    for c in range(N_CORES):
        mc = res_a.results[c]["means_flat"].reshape(SLOTS, C_IN)
        means[plan["seg_of"][c]] = mc

    nc_b = _build_b()
    in_map_b = {
        "means": means,
        "ident": np.eye(P, dtype=np.float32),
        "w1": np.asarray(inputs["W1"], dtype=np.float32),
        "w2": np.asarray(inputs["W2"], dtype=np.float32),
        "b1": np.asarray(inputs["b1"], dtype=np.float32).reshape(FC1, 1),
        "g1": np.asarray(inputs["g1"], dtype=np.float32).reshape(FC1, 1),
        "bt1": np.asarray(inputs["beta1"], dtype=np.float32).reshape(FC1, 1),
        "b2": np.asarray(inputs["b2"], dtype=np.float32).reshape(FC2, 1),
        "g2": np.asarray(inputs["g2"], dtype=np.float32).reshape(FC2, 1),
        "bt2": np.asarray(inputs["beta2"], dtype=np.float32).reshape(FC2, 1),
    }
    res_b = run_bass_kernel_spmd(nc_b, [in_map_b], [0], trace=trace)
    out = res_b.results[0]["out"].astype(np.float32)
    return out, {"res_a": res_a, "res_b": res_b}


def kernel(**inputs):
    return _run(inputs, trace=False)[0]


# revision 8
# speedup vs baseline: 1.2904x; 1.2904x over previous
"""Trainium2 Bass kernel for segment-mean + 2-layer MLP with training-mode BatchNorm.

Reference computation (see harness):
    ends = cumsum(length); seg_ids = searchsorted(ends, arange(N), 'right')
    mean  = segment_sum(x, seg_ids, B) / length[:, None]          # [512, 32]
    h   = relu(BN(mean @ W1 + b1, g1, beta1))                     # BN over batch dim
    out = BN(h @ W2 + b2, g2, beta2)                              # [512, 128]

Strategy (8 NeuronCores, full inputs in / full output out):
  Launch A (SPMD x8, memory-bound part):
    - 512 segments are rank-sorted by length and dealt into 64 "slots" x 8 cores
      (slot i holds the 8 segments ranked [8i, 8i+8); one per core), so every
      core runs the IDENTICAL program on a same-shape buffer.
    - Host packs each core's buffer [128, W]: slot i padded to L_i rows
      (L_i = per-slot max, multiple of 128, same on all cores); partition p
      holds rows [p*L_i/128, (p+1)*L_i/128) of the slot, so each slot occupies
      the same free-dim window on every partition.
    - Device: stream slot-aligned tiles (contiguous per-partition DMA), one
      strided VectorE reduce per slot ([128,(r c)] -> [128, 32]), then a
      single ones-vector TensorE matmul merges the 128 partitions into
      [1, 64*32], scaled by 1/len -> per-slot means.
  Launch B (1 core): tiny MLP+BN on the gathered [512, 32] means. Batch lives
    on the free axis (h^T layouts) so BN stats are free-axis reduces and the
    normalization is one fused scalar-engine activation per layer.

kernel() is self-contained: shapes/sharding hardcoded, no file reads.
"""

import sys

if "/opt/trn_rl_repo" not in sys.path:
    sys.path.insert(0, "/opt/trn_rl_repo")

import numpy as np

import concourse.bass as bass
import concourse.tile as tile
from concourse import bacc, mybir
from concourse.bass_utils import run_bass_kernel_spmd

F32 = mybir.dt.float32

N_TOTAL = 4_194_304
B = 512
C_IN = 32
FC1 = 64
FC2 = 128
EPS = 1e-5
N_CORES = 8
P = 128
SLOTS = B // N_CORES          # 64 slots per core
TILE_W = 8192                 # target free-dim elems per DMA tile (32 KiB/partition)


# ---------------------------------------------------------------- host layout

def _plan(lens):
    """Assign segments to (core, slot), pick padded slot lengths and DMA tiles.

    Returns dict with:
      seg_of[c][i] -> segment id
      li[i]        -> rows per partition for slot i (L_i = 128*li)
      w[i]         -> free-dim elems per slot (li*32)
      tiles        -> list of (offset, width, [(slot, off_in_tile, li), ...])
    """
    order = np.argsort(-lens, kind="stable")
    seg_of = np.empty((N_CORES, SLOTS), dtype=np.int64)
    li = np.empty(SLOTS, dtype=np.int64)
    for i in range(SLOTS):
        group = order[i * N_CORES:(i + 1) * N_CORES]
        seg_of[:, i] = group
        li[i] = (int(lens[group].max()) + P - 1) // P
    w = li * C_IN
    tiles = []
    cur = []
    cur_w = 0
    off = 0
    for i in range(SLOTS):
        if cur and cur_w + int(w[i]) > TILE_W:
            tiles.append((off, cur_w, cur))
            off += cur_w
            cur, cur_w = [], 0
        cur.append((i, cur_w, int(li[i])))
        cur_w += int(w[i])
    if cur:
        tiles.append((off, cur_w, cur))
    max_w = max(t[1] for t in tiles)
    return {"seg_of": seg_of, "li": li, "w": w, "W": int(w.sum()),
            "tiles": tiles, "max_w": max_w}


def _pack(x, lens, starts, plan):
    """Build per-core device buffers [128, W] plus per-core 1/len rows."""
    W = plan["W"]
    seg_of = plan["seg_of"]
    li = plan["li"]
    xbufs = []
    invs = []
    for c in range(N_CORES):
        buf = np.zeros((P, W), dtype=np.float32)
        off = 0
        for i in range(SLOTS):
            s = int(seg_of[c, i])
            L, wi = int(lens[s]), int(li[i] * C_IN)
            rows = np.zeros((P * int(li[i]), C_IN), dtype=np.float32)
            rows[:L] = x[starts[s]:starts[s] + L]
            # channel-major per partition so the device reduce streams stride-1
            chunk = rows.reshape(P, int(li[i]), C_IN).transpose(0, 2, 1)
            buf[:, off:off + wi] = chunk.reshape(P, wi)
            off += wi
        xbufs.append(buf)
        linv = (np.float32(1.0) / lens[seg_of[c]].astype(np.float32))
        invs.append(np.repeat(linv, C_IN)[None, :].astype(np.float32))
    return xbufs, invs


# ---------------------------------------------------------------- device progs

def _build_a(plan):
    """Launch A: per-core segment means -> [1, SLOTS*C_IN]."""
    W = plan["W"]
    nc = bacc.Bacc("TRN2", target_bir_lowering=False, debug=False)
    x_d = nc.dram_tensor("xd", [P, W], F32, kind="ExternalInput")
    inv_d = nc.dram_tensor("inv", [1, SLOTS * C_IN], F32, kind="ExternalInput")
    out_d = nc.dram_tensor("means_flat", [1, SLOTS * C_IN], F32, kind="ExternalOutput")

    with tile.TileContext(nc) as tc:
        with (
            tc.tile_pool(name="xin", bufs=3) as xin,
            tc.tile_pool(name="cons", bufs=1) as cons,
            tc.tile_pool(name="ps", bufs=1, space="PSUM") as ps,
        ):
            partials = cons.tile([P, SLOTS * C_IN], F32)
            ones = cons.tile([P, 1], F32)
            nc.vector.memset(ones[:], 1.0)
            inv = cons.tile([1, SLOTS * C_IN], F32)
            nc.sync.dma_start(inv[:], inv_d[:])

            for off, wt, slots in plan["tiles"]:
                t = xin.tile([P, plan["max_w"]], F32, tag="xtile")
                nc.sync.dma_start(t[:, :wt], x_d[:, off:off + wt])
                for i, soff, sli in slots:
                    src = t[:, soff:soff + sli * C_IN].rearrange(
                        "p (c r) -> p c r", r=sli)
                    nc.vector.reduce_sum(
                        partials[:, i * C_IN:(i + 1) * C_IN], src,
                        axis=mybir.AxisListType.X)

            acc = ps.tile([1, SLOTS * C_IN], F32)
            for k in range(0, SLOTS * C_IN, 512):
                ke = min(k + 512, SLOTS * C_IN)
                nc.tensor.matmul(acc[:, k:ke], ones[:], partials[:, k:ke],
                                 start=True, stop=True)
            means = cons.tile([1, SLOTS * C_IN], F32)
            nc.vector.tensor_mul(means[:], acc[:, :], inv[:])
            nc.sync.dma_start(out_d[:], means[:])
    nc.compile()
    return nc


def _bn_free_axis(nc, cons, sb, h_sb, n_par, n_free, b_t, g_t, bt_t, relu):
    """BN over the free axis of h_sb [n_par, n_free] (+ optional ReLU), in place.

    h_sb already contains (pre-activation + bias). Returns output tile.
    """
    zeros1 = cons.tile([P, 1], F32, tag="zeros1")
    nc.vector.memset(zeros1[:], 0.0)
    eps1 = cons.tile([P, 1], F32, tag="eps1")
    nc.vector.memset(eps1[:], EPS)

    rs = sb.tile([n_par, 1], F32, tag=f"rs{n_par}")
    sq = sb.tile([n_par, n_free], F32, tag=f"sq{n_par}")
    ssq = sb.tile([n_par, 1], F32, tag=f"ssq{n_par}")
    nc.scalar.activation(sq[:], h_sb[:], mybir.ActivationFunctionType.Square,
                         bias=zeros1[0:n_par, :], accum_out=ssq[:])
    nc.vector.reduce_sum(rs[:], h_sb[:], axis=mybir.AxisListType.X)

    mu = sb.tile([n_par, 1], F32, tag=f"mu{n_par}")
    nc.scalar.mul(mu[:], rs[:], 1.0 / n_free)
    var = sb.tile([n_par, 1], F32, tag=f"var{n_par}")
    nc.scalar.mul(var[:], ssq[:], 1.0 / n_free)
    musq = sb.tile([n_par, 1], F32, tag=f"musq{n_par}")
    nc.vector.tensor_mul(musq[:], mu[:], mu[:])
    nc.vector.tensor_sub(var[:], var[:], musq[:])
    std = sb.tile([n_par, 1], F32, tag=f"std{n_par}")
    nc.scalar.activation(std[:], var[:], mybir.ActivationFunctionType.Sqrt,
                         bias=eps1[0:n_par, :], scale=1.0)
    rstd = sb.tile([n_par, 1], F32, tag=f"rstd{n_par}")
    nc.vector.reciprocal(rstd[:], std[:])

    scale = sb.tile([n_par, 1], F32, tag=f"scale{n_par}")
    nc.vector.tensor_mul(scale[:], g_t[:], rstd[:])
    bias = sb.tile([n_par, 1], F32, tag=f"bias{n_par}")
    nc.vector.tensor_mul(bias[:], mu[:], scale[:])
    nc.vector.tensor_sub(bias[:], bt_t[:], bias[:])

    out = sb.tile([n_par, n_free], F32, tag=f"bnout{n_par}")
    func = (mybir.ActivationFunctionType.Relu if relu
            else mybir.ActivationFunctionType.Identity)
    # activation computes func(in*scale + bias): scale = g*rstd,
    # bias = beta - mu*g*rstd  ->  func(g*(h-mu)*rstd + beta)
    nc.scalar.activation(out[:], h_sb[:], func, bias=bias[:], scale=scale[:])
    return out


def _build_b():
    """Launch B: [512, 32] means -> MLP+BN -> [512, 128]."""
    nc = bacc.Bacc("TRN2", target_bir_lowering=False, debug=False)
    m_d = nc.dram_tensor("means", [B, C_IN], F32, kind="ExternalInput")
    id_d = nc.dram_tensor("ident", [P, P], F32, kind="ExternalInput")
    w1_d = nc.dram_tensor("w1", [C_IN, FC1], F32, kind="ExternalInput")
    w2_d = nc.dram_tensor("w2", [FC1, FC2], F32, kind="ExternalInput")
    b1_d = nc.dram_tensor("b1", [FC1, 1], F32, kind="ExternalInput")
    g1_d = nc.dram_tensor("g1", [FC1, 1], F32, kind="ExternalInput")
    bt1_d = nc.dram_tensor("bt1", [FC1, 1], F32, kind="ExternalInput")
    b2_d = nc.dram_tensor("b2", [FC2, 1], F32, kind="ExternalInput")
    g2_d = nc.dram_tensor("g2", [FC2, 1], F32, kind="ExternalInput")
    bt2_d = nc.dram_tensor("bt2", [FC2, 1], F32, kind="ExternalInput")
    out_d = nc.dram_tensor("out", [B, FC2], F32, kind="ExternalOutput")

    with tile.TileContext(nc) as tc:
        with (
            tc.tile_pool(name="cons", bufs=1) as cons,
            tc.tile_pool(name="sb", bufs=1) as sb,
            tc.tile_pool(name="pst", bufs=2, space="PSUM") as pst,
            tc.tile_pool(name="psm", bufs=1, space="PSUM") as psm,
        ):
            ident = cons.tile([P, P], F32)
            nc.sync.dma_start(ident[:], id_d[:])
            w1 = cons.tile([C_IN, FC1], F32)
            nc.sync.dma_start(w1[:], w1_d[:])
            w2 = cons.tile([FC1, FC2], F32)
            nc.sync.dma_start(w2[:], w2_d[:])
            b1 = cons.tile([FC1, 1], F32)
            nc.sync.dma_start(b1[:], b1_d[:])
            g1 = cons.tile([FC1, 1], F32)
            nc.sync.dma_start(g1[:], g1_d[:])
            bt1 = cons.tile([FC1, 1], F32)
            nc.sync.dma_start(bt1[:], bt1_d[:])
            b2 = cons.tile([FC2, 1], F32)
            nc.sync.dma_start(b2[:], b2_d[:])
            g2 = cons.tile([FC2, 1], F32)
            nc.sync.dma_start(g2[:], g2_d[:])
            bt2 = cons.tile([FC2, 1], F32)
            nc.sync.dma_start(bt2[:], bt2_d[:])

            # means^T [32, 512] via 4 tensor-engine transposes
            mt = sb.tile([C_IN, B], F32)
            for j in range(B // P):
                mj = sb.tile([P, C_IN], F32, tag="mj")
                nc.sync.dma_start(mj[:], m_d[j * P:(j + 1) * P, :])
                tp = pst.tile([C_IN, P], F32, tag="tp")
                nc.tensor.transpose(tp[:], mj[:], ident[:])
                nc.scalar.copy(mt[:, j * P:(j + 1) * P], tp[:])

            # layer 1: h1^T [64, 512] = W1^T @ means^T, +b1, BN, ReLU
            h1_ps = psm.tile([FC1, B], F32, tag="h1")
            nc.tensor.matmul(h1_ps[:], w1[:], mt[:], start=True, stop=True)
            h1 = sb.tile([FC1, B], F32)
            nc.scalar.activation(h1[:], h1_ps[:],
                                 mybir.ActivationFunctionType.Identity,
                                 bias=b1[:])
            a1 = _bn_free_axis(nc, cons, sb, h1, FC1, B, b1, g1, bt1,
                               relu=True)

            # layer 2: h2^T [128, 512] = W2^T @ a1, +b2, BN
            h2_ps = psm.tile([FC2, B], F32, tag="h2")
            nc.tensor.matmul(h2_ps[:], w2[:], a1[:], start=True, stop=True)
            h2 = sb.tile([FC2, B], F32)
            nc.scalar.activation(h2[:], h2_ps[:],
                                 mybir.ActivationFunctionType.Identity,
                                 bias=b2[:])
            o = _bn_free_axis(nc, cons, sb, h2, FC2, B, b2, g2, bt2,
                              relu=False)

            # transpose back to [512, 128] and store
            for j in range(B // P):
                tp2 = pst.tile([P, P], F32, tag="tp2")
                nc.tensor.transpose(tp2[:], o[:, j * P:(j + 1) * P], ident[:])
                ob = sb.tile([P, P], F32, tag="ob")
                nc.scalar.copy(ob[:], tp2[:])
                nc.sync.dma_start(out_d[j * P:(j + 1) * P, :], ob[:])
    nc.compile()
    return nc


# ---------------------------------------------------------------- entry point

def _run(inputs, trace=False):
    x = np.ascontiguousarray(np.asarray(inputs["x"], dtype=np.float32))
    lens = np.asarray(inputs["length"]).astype(np.int64)
    starts = np.zeros(B + 1, dtype=np.int64)
    np.cumsum(lens, out=starts[1:])
    assert starts[-1] == x.shape[0]

    plan = _plan(lens)
    xbufs, invs = _pack(x, lens, starts, plan)

    nc_a = _build_a(plan)
    in_maps = [{"xd": xbufs[c], "inv": invs[c]} for c in range(N_CORES)]
    res_a = run_bass_kernel_spmd(nc_a, in_maps, list(range(N_CORES)),
                                 trace=trace)

    means = np.empty((B, C_IN), dtype=np.float32)
    for c in range(N_CORES):
        mc = res_a.results[c]["means_flat"].reshape(SLOTS, C_IN)
        means[plan["seg_of"][c]] = mc

    nc_b = _build_b()
    in_map_b = {
        "means": means,
        "ident": np.eye(P, dtype=np.float32),
        "w1": np.asarray(inputs["W1"], dtype=np.float32),
        "w2": np.asarray(inputs["W2"], dtype=np.float32),
        "b1": np.asarray(inputs["b1"], dtype=np.float32).reshape(FC1, 1),
        "g1": np.asarray(inputs["g1"], dtype=np.float32).reshape(FC1, 1),
        "bt1": np.asarray(inputs["beta1"], dtype=np.float32).reshape(FC1, 1),
        "b2": np.asarray(inputs["b2"], dtype=np.float32).reshape(FC2, 1),
        "g2": np.asarray(inputs["g2"], dtype=np.float32).reshape(FC2, 1),
        "bt2": np.asarray(inputs["beta2"], dtype=np.float32).reshape(FC2, 1),
    }
    res_b = run_bass_kernel_spmd(nc_b, [in_map_b], [0], trace=trace)
    out = res_b.results[0]["out"].astype(np.float32)
    return out, {"res_a": res_a, "res_b": res_b}


def kernel(**inputs):
    return _run(inputs, trace=False)[0]
